# revision 11
# baseline (speedup 1.0000x reference)
"""Trainium2 Bass kernel for a 6-layer transformer decoder (nn_Decoder_trans).

Sharding: 8 cores = 4 batch pairs x 2-way sequence split (512 rows/core).
Self-attention K/V requires the full 1024-row sequence, so each layer does one
bf16 AllGather of the (feature-major) residual stream within each core pair.

On-chip layout is feature-major ([features(part), rows(free)]) so projection
matmuls need no transposes: out = W.T @ X^T with W as the stationary operand.
Weights are pre-cast to bf16 on the host. LayerNorm stats use PE ones-matmul
column reductions; [1,N] -> [128,N] broadcasts use PE ones outer-products.
Softmax denominators ride along the AV matmul as an extra ones-column of V.
"""

import numpy as np
import ml_dtypes

import concourse.bass as bass
import concourse.mybir as mybir
import concourse.tile as tile
from concourse import bacc
from concourse.bass_utils import run_bass_kernel_spmd

f32 = mybir.dt.float32
bf16 = mybir.dt.bfloat16
AF = mybir.ActivationFunctionType
ALU = mybir.AluOpType

B, SQ, SK = 4, 1024, 1024
IN, H, NH, PF, L = 128, 1024, 16, 4096, 6
HD = H // NH
SCALE = float(np.sqrt(H))
EPS = 1e-5
R = 512            # rows per core
FT = H // 128      # 8 feature tiles
KT = SK // 128     # 8 key tiles
PFT = PF // 128    # 32 pf tiles


def _build():
    nc = bacc.Bacc(trn_type="TRN2", num_devices=8)

    # ---- DRAM I/O ----
    dec_xT = nc.dram_tensor("dec_xT", [IN, R], bf16, kind="ExternalInput")
    pos_eT = nc.dram_tensor("pos_eT", [H, R], f32, kind="ExternalInput")
    encT = nc.dram_tensor("encT", [H, SK], bf16, kind="ExternalInput")
    maskT = nc.dram_tensor("maskT", [SK, R], bf16, kind="ExternalInput")
    emb_w = nc.dram_tensor("emb_w", [IN, H], bf16, kind="ExternalInput")
    emb_b = nc.dram_tensor("emb_b", [H], f32, kind="ExternalInput")  # pre-scaled x32
    attn_w = nc.dram_tensor("attn_w", [L, 2, 4, H, H], bf16, kind="ExternalInput")
    attn_b = nc.dram_tensor("attn_b", [L, 2, 4, H], f32, kind="ExternalInput")
    attn_vb = nc.dram_tensor("attn_vb", [L, 2, H], bf16, kind="ExternalInput")
    ln_g = nc.dram_tensor("ln_g", [L, 3, H], f32, kind="ExternalInput")
    ln_b = nc.dram_tensor("ln_b", [L, 3, H], f32, kind="ExternalInput")
    ff_w1 = nc.dram_tensor("ff_w1", [L, H, PF], bf16, kind="ExternalInput")
    ff_b1 = nc.dram_tensor("ff_b1", [L, PF], f32, kind="ExternalInput")
    ff_w2 = nc.dram_tensor("ff_w2", [L, PF, H], bf16, kind="ExternalInput")
    ff_b2 = nc.dram_tensor("ff_b2", [L, H], f32, kind="ExternalInput")
    out_w = nc.dram_tensor("out_w", [H, IN], bf16, kind="ExternalInput")
    out_b = nc.dram_tensor("out_b", [IN], f32, kind="ExternalInput")
    outT = nc.dram_tensor("outT", [IN, R], f32, kind="ExternalOutput")
    aprobs = nc.dram_tensor("aprobs", [NH, R, SK], f32, kind="ExternalOutput")

    with tile.TileContext(nc) as tc:
        import contextlib
        ctx = contextlib.ExitStack()
        persist = ctx.enter_context(tc.tile_pool(name="persist", bufs=1))
        wpool = ctx.enter_context(tc.tile_pool(name="wpool", bufs=9))
        act = ctx.enter_context(tc.tile_pool(name="act", bufs=2))
        small = ctx.enter_context(tc.tile_pool(name="small", bufs=3))
        probsp = ctx.enter_context(tc.tile_pool(name="probsp", bufs=3))
        dram = ctx.enter_context(tc.tile_pool(name="dram", bufs=2, space="DRAM"))
        ps_big = ctx.enter_context(tc.tile_pool(name="ps_big", bufs=2, space="PSUM"))
        ps_sc = ctx.enter_context(tc.tile_pool(name="ps_sc", bufs=2, space="PSUM"))
        ps_av = ctx.enter_context(tc.tile_pool(name="ps_av", bufs=1, space="PSUM"))
        ps_bc = ctx.enter_context(tc.tile_pool(name="ps_bc", bufs=1, space="PSUM"))
        ps_f2 = ctx.enter_context(tc.tile_pool(name="ps_f2", bufs=2, space="PSUM"))

        # ---- persistent on-chip tensors ----
        ones = persist.tile([128, 128], bf16)
        nc.vector.memset(ones, 1.0)
        eps_t = persist.tile([1, 1], f32)
        nc.vector.memset(eps_t, EPS)
        enc_t = persist.tile([128, FT, SK], bf16)
        nc.sync.dma_start(out=enc_t, in_=encT[:].rearrange("(t p) k -> p t k", p=128))
        mask_t = persist.tile([128, KT, R], bf16)
        nc.sync.dma_start(out=mask_t, in_=maskT[:].rearrange("(t p) r -> p t r", p=128))

        def load_w_tiles(dram_ap_2d, n_k, cols):
            """Load [n_k*128, cols] weight matrix as n_k tiles of [128, cols]."""
            ts = []
            for k in range(n_k):
                t = wpool.tile([128, cols], bf16, tag="w")
                nc.sync.dma_start(out=t, in_=dram_ap_2d[k * 128:(k + 1) * 128, :])
                ts.append(t)
            return ts

        def load_bias_col(dram_ap_1d, n_m, tag="bias"):
            t = small.tile([128, n_m], f32, tag=tag)
            nc.sync.dma_start(out=t, in_=dram_ap_1d.rearrange("(m p) -> p m", p=128))
            return t

        # ---- embedding ----
        embw_t = load_w_tiles(emb_w[:], 1, H)[0]  # [128, 1024]
        embb_t = load_bias_col(emb_b[:], FT)
        dec_t = persist.tile([128, R], bf16)
        nc.sync.dma_start(out=dec_t, in_=dec_xT[:])
        pos_t = act.tile([128, FT, R], f32, tag="relu", bufs=1)
        nc.sync.dma_start(out=pos_t, in_=pos_eT[:].rearrange("(t p) r -> p t r", p=128))

        trg = act.tile([128, FT, R], f32, tag="trg")
        trg_bf = act.tile([128, FT, R], bf16, tag="trgbf")
        for m in range(FT):
            ps = ps_big.tile([128, R], f32, tag="proj")
            nc.tensor.matmul(out=ps, lhsT=embw_t[:, m * 128:(m + 1) * 128],
                             rhs=dec_t, start=True, stop=True)
            nc.scalar.activation(out=trg[:, m, :], in_=ps, func=AF.Identity,
                                 scale=SCALE, bias=embb_t[:, m:m + 1])
            nc.vector.tensor_add(trg[:, m, :], trg[:, m, :], pos_t[:, m, :])
            nc.vector.tensor_copy(out=trg_bf[:, m, :], in_=trg[:, m, :])

        def layernorm(x, g_col, b_col):
            """x: [128, FT, R] f32 (consumed in-place). Returns (trg, trg_bf)."""
            xbf = act.tile([128, FT, R], bf16, tag="q", bufs=1)
            for k in range(FT):
                nc.vector.tensor_copy(out=xbf[:, k, :], in_=x[:, k, :])
            sum_ps = ps_sc.tile([1, R], f32, tag="sc")
            for k in range(FT):
                nc.tensor.matmul(out=sum_ps, lhsT=ones[:, 0:1], rhs=xbf[:, k, :],
                                 start=(k == 0), stop=(k == FT - 1))
            # in-place square after sums consumed xbf
            for k in range(FT):
                nc.vector.tensor_mul(xbf[:, k, :], xbf[:, k, :], xbf[:, k, :])
            ssq_ps = ps_sc.tile([1, R], f32, tag="sc")
            for k in range(FT):
                nc.tensor.matmul(out=ssq_ps, lhsT=ones[:, 0:1], rhs=xbf[:, k, :],
                                 start=(k == 0), stop=(k == FT - 1))
            mu = small.tile([1, R], f32, tag="lnrow", bufs=2)
            nc.vector.tensor_scalar_mul(mu, sum_ps, 1.0 / H)
            negmu_bf = small.tile([1, R], bf16, tag="lnrowbf", bufs=2)
            nc.vector.tensor_scalar_mul(negmu_bf, sum_ps, -1.0 / H)
            musq = small.tile([1, R], f32, tag="lnrow", bufs=2)
            nc.vector.tensor_mul(musq, mu, mu)
            var = small.tile([1, R], f32, tag="lnrow", bufs=2)
            nc.vector.scalar_tensor_tensor(out=var, in0=ssq_ps, scalar=1.0 / H,
                                           in1=musq, op0=ALU.mult, op1=ALU.subtract)
            sd = small.tile([1, R], f32, tag="lnrow", bufs=2)
            nc.scalar.activation(out=sd, in_=var, func=AF.Sqrt, bias=eps_t)
            rsig_bf = small.tile([1, R], bf16, tag="lnrowbf", bufs=2)
            with nc.allow_low_precision(reason="bf16 rsig row for PE broadcast"):
                nc.vector.reciprocal(out=rsig_bf, in_=sd)
            negmu_ps = ps_sc.tile([128, R], f32, tag="sc")
            nc.tensor.matmul(out=negmu_ps, lhsT=ones[0:1, :], rhs=negmu_bf,
                             start=True, stop=True)
            rsig_ps = ps_sc.tile([128, R], f32, tag="sc")
            nc.tensor.matmul(out=rsig_ps, lhsT=ones[0:1, :], rhs=rsig_bf,
                             start=True, stop=True)
            y = act.tile([128, FT, R], f32, tag="trg")
            ybf = act.tile([128, FT, R], bf16, tag="trgbf")
            for k in range(FT):
                nc.vector.tensor_add(x[:, k, :], x[:, k, :], negmu_ps)
                nc.vector.tensor_mul(x[:, k, :], x[:, k, :], rsig_ps)
                nc.scalar.activation(out=y[:, k, :], in_=x[:, k, :], func=AF.Identity,
                                     scale=g_col[:, k:k + 1], bias=b_col[:, k:k + 1])
                nc.vector.tensor_copy(out=ybf[:, k, :], in_=y[:, k, :])
            return y, ybf

        def attention(li, ai, q_src_bf, kv_chunk, kv_ktile, resid, use_mask,
                      emit_probs=False):
            """One MHA sublayer. q_src_bf: [128, FT, R] bf16. kv_chunk(f, c)
            -> [128, 512] keys chunk; kv_ktile(f, kt) -> [128, 128].
            Returns x = resid + o_proj(attn) as f32 [128, FT, R] (in resid's slot).
            """
            wq = load_w_tiles(attn_w[li, ai, 0], FT, H)
            bq = load_bias_col(attn_b[li, ai, 0], FT, tag="bq")
            q_bf = act.tile([128, FT, R], bf16, tag="q", bufs=1)
            for m in range(FT):
                ps = ps_big.tile([128, R], f32, tag="proj")
                for k in range(FT):
                    nc.tensor.matmul(out=ps, lhsT=wq[k][:, m * 128:(m + 1) * 128],
                                     rhs=q_src_bf[:, k, :],
                                     start=(k == 0), stop=(k == FT - 1))
                nc.scalar.activation(out=q_bf[:, m, :], in_=ps, func=AF.Identity,
                                     bias=bq[:, m:m + 1])

            wk = load_w_tiles(attn_w[li, ai, 1], FT, H)
            bk = load_bias_col(attn_b[li, ai, 1], FT, tag="bk")
            k_bf = act.tile([128, FT, SK], bf16, tag="k", bufs=1)
            for m in range(FT):
                for c in range(SK // 512):
                    ps = ps_big.tile([128, 512], f32, tag="proj")
                    for k in range(FT):
                        nc.tensor.matmul(out=ps, lhsT=wk[k][:, m * 128:(m + 1) * 128],
                                         rhs=kv_chunk(k, c),
                                         start=(k == 0), stop=(k == FT - 1))
                    nc.scalar.activation(out=k_bf[:, m, c * 512:(c + 1) * 512], in_=ps,
                                         func=AF.Identity, bias=bk[:, m:m + 1])

            wv = load_w_tiles(attn_w[li, ai, 2], FT, H)
            vb_row = small.tile([1, H], bf16, tag="vbrow", bufs=1)
            nc.sync.dma_start(out=vb_row, in_=attn_vb[li, ai].unsqueeze(0))
            v_bf = act.tile([128, KT, NH, HD + 1], bf16, tag="v", bufs=1)
            for kt in range(KT):
                nc.vector.memset(v_bf[:, kt, :, HD:HD + 1], 1.0)
                for c in range(2):
                    ps = ps_big.tile([128, 512], f32, tag="proj")
                    for k in range(FT):
                        nc.tensor.matmul(out=ps, lhsT=kv_ktile(k, kt),
                                         rhs=wv[k][:, c * 512:(c + 1) * 512],
                                         start=(k == 0), stop=False)
                    nc.tensor.matmul(out=ps, lhsT=ones[0:1, :],
                                     rhs=vb_row[:, c * 512:(c + 1) * 512],
                                     start=False, stop=True)
                    nc.scalar.activation(
                        out=v_bf[:, kt, 8 * c:8 * (c + 1), 0:HD],
                        in_=ps.rearrange("p (h d) -> p h d", d=HD), func=AF.Copy)

            attnout = act.tile([128, FT, R], bf16, tag="attnout", bufs=1)
            for h in range(NH):
                base = 64 * (h % 2)
                ft = h // 2
                av = ps_av.tile([HD + 1, R], f32, tag="av", bufs=1)
                for kt in range(KT):
                    s_ps = ps_sc.tile([128, R], f32, tag="sc")
                    nc.tensor.matmul(
                        out=s_ps,
                        lhsT=k_bf[base:base + 64, ft, kt * 128:(kt + 1) * 128],
                        rhs=q_bf[base:base + 64, ft, :], start=True, stop=True)
                    p_bf = probsp.tile([128, R], bf16, tag="probs")
                    nc.scalar.activation(out=p_bf, in_=s_ps, func=AF.Exp,
                                         scale=1.0 / SCALE)
                    if use_mask:
                        nc.vector.tensor_mul(p_bf, p_bf, mask_t[:, kt, :])
                    nc.tensor.matmul(out=av, lhsT=v_bf[:, kt, h, :], rhs=p_bf,
                                     start=(kt == 0), stop=(kt == KT - 1))
                rc_bf = small.tile([HD + 1, R], bf16, tag="recip", bufs=2)
                with nc.allow_low_precision(reason="bf16 softmax recip for PE broadcast"):
                    nc.vector.reciprocal(out=rc_bf[HD:HD + 1, :], in_=av[HD:HD + 1, :])
                bc_ps = ps_bc.tile([64, R], f32, tag="avbc", bufs=1)
                nc.tensor.matmul(out=bc_ps, lhsT=ones[HD:HD + 1, 0:64],
                                 rhs=rc_bf[HD:HD + 1, :], start=True, stop=True)
                avs = small.tile([64, R], bf16, tag="avsb", bufs=2)
                nc.scalar.activation(out=avs, in_=av[0:64, :], func=AF.Copy)
                if base == 0:
                    nc.vector.tensor_mul(attnout[0:64, ft, :], avs, bc_ps)
                else:
                    tmp = small.tile([64, R], bf16, tag="avtmp", bufs=2)
                    nc.vector.tensor_mul(tmp, avs, bc_ps)
                    nc.sync.dma_start(out=attnout[64:128, ft, :], in_=tmp)

            if emit_probs:
                # row-major recompute of this attention's probs -> aprobs output
                for h in range(NH):
                    base = 64 * (h % 2)
                    ft = h // 2
                    for qt in range(4):
                        pr = probsp.tile([128, SK], f32, tag="prout", bufs=2)
                        den = small.tile([128, 2], f32, tag="den")
                        for c in range(2):
                            s2 = ps_sc.tile([128, 512], f32, tag="sc")
                            nc.tensor.matmul(
                                out=s2,
                                lhsT=q_bf[base:base + 64, ft, qt * 128:(qt + 1) * 128],
                                rhs=k_bf[base:base + 64, ft, c * 512:(c + 1) * 512],
                                start=True, stop=True)
                            nc.scalar.activation(out=pr[:, c * 512:(c + 1) * 512],
                                                 in_=s2, func=AF.Exp, scale=1.0 / SCALE,
                                                 accum_out=den[:, c:c + 1])
                        dsum = small.tile([128, 1], f32, tag="dsum")
                        nc.vector.tensor_add(dsum, den[:, 0:1], den[:, 1:2])
                        rcp = small.tile([128, 1], f32, tag="rcp")
                        nc.vector.reciprocal(out=rcp, in_=dsum)
                        nc.vector.tensor_scalar_mul(pr, pr, rcp)
                        nc.sync.dma_start(
                            out=aprobs[h, qt * 128:(qt + 1) * 128, :], in_=pr)

            wo = load_w_tiles(attn_w[li, ai, 3], FT, H)
            bo = load_bias_col(attn_b[li, ai, 3], FT, tag="bo")
            for m in range(FT):
                ps = ps_big.tile([128, R], f32, tag="proj")
                for k in range(FT):
                    nc.tensor.matmul(out=ps, lhsT=wo[k][:, m * 128:(m + 1) * 128],
                                     rhs=attnout[:, k, :],
                                     start=(k == 0), stop=(k == FT - 1))
                nc.vector.scalar_tensor_tensor(
                    out=resid[:, m, :], in0=ps, scalar=bo[:, m:m + 1],
                    in1=resid[:, m, :], op0=ALU.add, op1=ALU.add)
            return resid

        # ---- layers ----
        for li in range(L):
            # gather trg across the pair for self-attn K/V
            g_in = dram.tile([H, R], bf16, tag="gin")
            g_out = dram.tile([2 * H, R], bf16, tag="gout")
            nc.sync.dma_start(out=g_in[:].rearrange("(t p) r -> p t r", p=128),
                              in_=trg_bf)
            nc.gpsimd.collective_compute(
                "AllGather", ALU.bypass,
                replica_groups=[[0, 1], [2, 3], [4, 5], [6, 7]],
                ins=[g_in.opt()], outs=[g_out.opt()])
            gath = act.tile([128, FT, 2, 512], bf16, tag="relu", bufs=1)
            for bb in range(2):
                nc.sync.dma_start(
                    out=gath[:, :, bb, :],
                    in_=g_out[bb * H:(bb + 1) * H, :].rearrange(
                        "(t p) r -> p t r", p=128))

            def self_chunk(f, c):
                return gath[:, f, c, :]

            def self_ktile(f, kt):
                return gath[:, f, kt // 4, (kt % 4) * 128:(kt % 4 + 1) * 128]

            def enc_chunk(f, c):
                return enc_t[:, f, c * 512:(c + 1) * 512]

            def enc_ktile(f, kt):
                return enc_t[:, f, kt * 128:(kt + 1) * 128]

            g1 = load_bias_col(ln_g[li, 0], FT, tag="lng")
            b1 = load_bias_col(ln_b[li, 0], FT, tag="lnb")
            x = attention(li, 0, trg_bf, self_chunk, self_ktile, trg, use_mask=True)
            trg, trg_bf = layernorm(x, g1, b1)

            g2 = load_bias_col(ln_g[li, 1], FT, tag="lng")
            b2 = load_bias_col(ln_b[li, 1], FT, tag="lnb")
            x = attention(li, 1, trg_bf, enc_chunk, enc_ktile, trg, use_mask=False,
                          emit_probs=(li == L - 1))
            trg, trg_bf = layernorm(x, g2, b2)

            # ---- FFN ----
            b1c = load_bias_col(ff_b1[li], PFT, tag="fb1")
            relu = act.tile([128, PFT, 512], bf16, tag="relu", bufs=1)
            for quarter in range(4):
                w1q = []
                for k in range(FT):
                    t = wpool.tile([128, PF // 4], bf16, tag="w")
                    nc.sync.dma_start(
                        out=t, in_=ff_w1[li][k * 128:(k + 1) * 128,
                                             quarter * 1024:(quarter + 1) * 1024])
                    w1q.append(t)
                for pfq in range(8):
                    pf = quarter * 8 + pfq
                    ps = ps_big.tile([128, R], f32, tag="proj")
                    for k in range(FT):
                        nc.tensor.matmul(
                            out=ps, lhsT=w1q[k][:, pfq * 128:(pfq + 1) * 128],
                            rhs=trg_bf[:, k, :], start=(k == 0), stop=(k == FT - 1))
                    nc.scalar.activation(out=relu[:, pf, :], in_=ps, func=AF.Relu,
                                         bias=b1c[:, pf:pf + 1])
            b2c = load_bias_col(ff_b2[li], FT, tag="fb2")
            x = trg  # residual written in place
            for grp in range(2):
                pss = []
                for _i in range(2):
                    f2ps = ps_f2.tile([128, R], f32, tag="f2", bufs=2)
                    pss.append(f2ps)
                for _i in range(2):
                    f2ps2 = ps_big.tile([128, R], f32, tag="proj", bufs=2)
                    pss.append(f2ps2)
                for pf in range(PFT):
                    w2t = wpool.tile([128, 512], bf16, tag="w")
                    nc.sync.dma_start(
                        out=w2t, in_=ff_w2[li][pf * 128:(pf + 1) * 128,
                                               grp * 512:(grp + 1) * 512])
                    for m in range(4):
                        nc.tensor.matmul(out=pss[m], lhsT=w2t[:, m * 128:(m + 1) * 128],
                                         rhs=relu[:, pf, :],
                                         start=(pf == 0), stop=(pf == PFT - 1))
                for m in range(4):
                    mm = grp * 4 + m
                    nc.vector.scalar_tensor_tensor(
                        out=x[:, mm, :], in0=pss[m], scalar=b2c[:, mm:mm + 1],
                        in1=x[:, mm, :], op0=ALU.add, op1=ALU.add)
            g3 = load_bias_col(ln_g[li, 2], FT, tag="lng")
            b3 = load_bias_col(ln_b[li, 2], FT, tag="lnb")
            trg, trg_bf = layernorm(x, g3, b3)

        # ---- final projection ----
        ow_t = persist.tile([128, FT, IN], bf16)
        nc.sync.dma_start(out=ow_t, in_=out_w[:].rearrange("(k p) m -> p k m", p=128))
        ob_t = load_bias_col(out_b[:], 1, tag="ob")
        ps = ps_big.tile([128, R], f32, tag="proj")
        for k in range(FT):
            nc.tensor.matmul(out=ps, lhsT=ow_t[:, k, :], rhs=trg_bf[:, k, :],
                             start=(k == 0), stop=(k == FT - 1))
        fin = act.tile([128, R], f32, tag="q", bufs=1)
        nc.scalar.activation(out=fin, in_=ps, func=AF.Identity, bias=ob_t[:, 0:1])
        nc.sync.dma_start(out=outT[:], in_=fin)

        ctx.close()

    nc.compile()
    return nc


_NC = None


def _get_nc():
    global _NC
    if _NC is None:
        _NC = _build()
    return _NC


def _prep_inputs(dec_x, enc_x, dec_mask, enc_mask, emb_w, emb_b, pos_e, attn_w,
                 attn_b, ln_g, ln_b, ff_w1, ff_b1, ff_w2, ff_b2, out_w, out_b):
    bf = ml_dtypes.bfloat16
    f = np.float32
    dec_x = np.asarray(dec_x, f)
    enc_x = np.asarray(enc_x, f)
    dec_mask = np.asarray(dec_mask)
    attn_w_bf = np.asarray(attn_w, f).astype(bf)
    attn_b_f = np.ascontiguousarray(np.asarray(attn_b, f))
    attn_vb = np.ascontiguousarray(attn_b_f[:, :, 2, :]).astype(bf)
    shared = {
        "emb_w": np.asarray(emb_w, f).astype(bf),
        "emb_b": np.ascontiguousarray(np.asarray(emb_b, f) * SCALE),
        "attn_w": attn_w_bf,
        "attn_b": attn_b_f,
        "attn_vb": attn_vb,
        "ln_g": np.ascontiguousarray(np.asarray(ln_g, f)),
        "ln_b": np.ascontiguousarray(np.asarray(ln_b, f)),
        "ff_w1": np.asarray(ff_w1, f).astype(bf),
        "ff_b1": np.ascontiguousarray(np.asarray(ff_b1, f)),
        "ff_w2": np.asarray(ff_w2, f).astype(bf),
        "ff_b2": np.ascontiguousarray(np.asarray(ff_b2, f)),
        "out_w": np.asarray(out_w, f).astype(bf),
        "out_b": np.ascontiguousarray(np.asarray(out_b, f)),
    }
    pos_full = np.asarray(pos_e, f)
    mask_full = np.asarray(dec_mask[0, 0], f)  # [SQ, SK]
    in_maps = []
    for c in range(8):
        b, half = c // 2, c % 2
        rows = slice(half * R, (half + 1) * R)
        m = dict(shared)
        m["dec_xT"] = np.ascontiguousarray(dec_x[b, rows, :].T).astype(bf)
        m["pos_eT"] = np.ascontiguousarray(pos_full[rows, :].T)
        m["encT"] = np.ascontiguousarray(enc_x[b].T).astype(bf)
        m["maskT"] = np.ascontiguousarray(mask_full[rows, :].T).astype(bf)
        in_maps.append(m)
    return in_maps


def _run(inputs, **run_kwargs):
    nc = _get_nc()
    in_maps = _prep_inputs(**inputs)
    res = run_bass_kernel_spmd(nc, in_maps, core_ids=list(range(8)), **run_kwargs)
    output = np.zeros((B, SQ, IN), np.float32)
    attention = np.zeros((B, NH, SQ, SK), np.float32)
    for c in range(8):
        b, half = c // 2, c % 2
        rows = slice(half * R, (half + 1) * R)
        output[b, rows, :] = res.results[c]["outT"].T
        attention[b, :, rows, :] = res.results[c]["aprobs"]
    return output, attention, res


def kernel(**inputs):
    output, attention, _ = _run(inputs)
    return output, attention
